# revision 4
# baseline (speedup 1.0000x reference)
"""Trainium2 Bass kernel for StyleGAN2-style modulated conv2d (ModConv2D).

Reference computation (per sample b):
    w      = kernel * (style[b] + 1)            # modulate   [3,3,Cin,Cout]
    w      = w / sqrt(sum(w^2, (kh,kw,Cin)) + 1e-8)  # demodulate per Cout
    y[b]   = conv2d_same(x[b], w)

Sharding: data-parallel over batch — 16 samples across 8 NeuronCores,
2 samples per core; the base kernel is replicated.

Device algorithm per core (2 samples):
  - conv as 9-tap accumulated matmuls: psum[cout,pix] += w[t,cin,cout]^T @
    xT[cin, pix+off], with x held zero-padded (66x66) and channel-major in
    SBUF (bf16), weights modulated on-chip (bf16), accumulation in fp32.
  - demod factor d[cout] = rsqrt(sum_cin s^2 * K2 + 1e-8) computed in fp32 on
    device (K2 = sum_t kernel^2 precomputed once per core), applied as a
    per-partition scale when evicting psum.
  - x is transposed to channel-major via PE transposes; output transposed
    back to pixel-major via PE transposes; all I/O tensors keep the
    reference layouts in HBM.
"""

import numpy as np

B, H, W, CIN, COUT, KH, KW = 16, 64, 64, 256, 256, 3, 3
NCORES = 8
BPC = B // NCORES  # samples per core
T = KH * KW  # 9 taps
HW = H * W  # 4096 pixels
PADW = W + 2  # 66

_CACHE = {}
LAST_EXEC_NS = None
LAST_MEAN_EXEC_NS = None


def _build_nc():
    from contextlib import ExitStack

    import concourse.bacc as bacc
    import concourse.bass as bass
    import concourse.mybir as mybir
    import concourse.tile as tile
    from concourse.masks import make_identity

    f32 = mybir.dt.float32
    bf16 = mybir.dt.bfloat16
    AF = mybir.ActivationFunctionType

    nc = bacc.Bacc("TRN2", target_bir_lowering=False, debug=False)

    x_d = nc.dram_tensor("x", [BPC, H, W, CIN], f32, kind="ExternalInput")
    s_d = nc.dram_tensor("style", [BPC, CIN], f32, kind="ExternalInput")
    k_d = nc.dram_tensor("kernel", [KH, KW, CIN, COUT], f32, kind="ExternalInput")
    y_d = nc.dram_tensor("y", [BPC, H, W, COUT], f32, kind="ExternalOutput")

    # element strides
    XB, XH, XWS = H * W * CIN, W * CIN, CIN  # x/y strides (same shapes)
    KKH, KKW, KCIN = KW * CIN * COUT, CIN * COUT, COUT

    def x_blk_ap(b, t8):
        # [128 pix, 4 sblk, 256 cin] starting at pixel (t8*4)*128
        off = b * XB + t8 * 4 * 128 * CIN
        return bass.AP(x_d, off, [[CIN, 128], [128 * CIN, 4], [1, CIN]])

    def y_blk_ap(b, t8):
        off = b * XB + t8 * 4 * 128 * COUT
        return bass.AP(y_d, off, [[COUT, 128], [128 * COUT, 4], [1, COUT]])

    def k_cc_ap(cc):
        # [128 cin, 9 taps, 256 cout]
        off = cc * 128 * COUT
        return bass.AP(k_d, off, [[COUT, 128], [KKW, T], [1, COUT]])

    with tile.TileContext(nc) as tc, ExitStack() as ctx:
        singles = ctx.enter_context(tc.tile_pool(name="singles", bufs=1))
        tmp_pool = ctx.enter_context(tc.tile_pool(name="tmp", bufs=1))
        wpool = ctx.enter_context(tc.tile_pool(name="wpool", bufs=2))
        dpool = ctx.enter_context(tc.tile_pool(name="dpool", bufs=2))
        srow_pool = ctx.enter_context(tc.tile_pool(name="srow", bufs=2))
        xpool = ctx.enter_context(tc.tile_pool(name="xpool", bufs=2))
        xtpool = ctx.enter_context(tc.tile_pool(name="xt", bufs=3))
        ospool = ctx.enter_context(tc.tile_pool(name="osb", bufs=3))
        obpool = ctx.enter_context(tc.tile_pool(name="ob", bufs=3))
        pconv = ctx.enter_context(tc.tile_pool(name="pconv", bufs=2, space="PSUM"))
        pxt = ctx.enter_context(tc.tile_pool(name="pxt", bufs=2, space="PSUM"))
        pot = ctx.enter_context(tc.tile_pool(name="pot", bufs=2, space="PSUM"))
        psmall = ctx.enter_context(tc.tile_pool(name="psmall", bufs=2, space="PSUM"))

        ident_b = singles.tile([128, 128], bf16)
        make_identity(nc, ident_b)
        ident_f = singles.tile([128, 128], f32)
        make_identity(nc, ident_f)
        ones1 = singles.tile([1, 1], f32)
        nc.vector.memset(ones1, 1.0)
        eps_sb = singles.tile([128, 1], f32)
        nc.vector.memset(eps_sb, 1e-8)

        # base kernel, channel-major: [128 cin, cc, 9, 256 cout]
        kbase = singles.tile([128, 2, T, COUT], f32)
        for cc in range(2):
            nc.sync.dma_start(out=kbase[:, cc], in_=k_cc_ap(cc))

        # K2[cin, cout] = sum_t kernel^2  (once per core)
        k2 = singles.tile([128, 2, COUT], f32)
        for cc in range(2):
            k2tmp = tmp_pool.tile([128, T, COUT], f32)
            nc.vector.tensor_mul(k2tmp, kbase[:, cc], kbase[:, cc])
            nc.vector.reduce_sum(
                out=k2[:, cc],
                in_=k2tmp.rearrange("p t c -> p c t"),
                axis=mybir.AxisListType.X,
            )

        for b in range(BPC):
            # ---- modulation + demodulation factors ----
            srow = srow_pool.tile([1, CIN], f32)
            nc.sync.dma_start(out=srow, in_=s_d.ap()[b : b + 1, :])
            srow1 = srow_pool.tile([1, CIN], f32)
            nc.vector.tensor_scalar_add(srow1, srow, 1.0)

            smod = dpool.tile([128, 2], f32)  # (style+1) col-major per cc
            s2c = dpool.tile([128, 2], f32)
            for cc in range(2):
                pcol = psmall.tile([128, 1], f32, tag="psmall")
                nc.tensor.matmul(
                    pcol, srow1[:, cc * 128 : (cc + 1) * 128], ones1, start=True, stop=True
                )
                nc.vector.tensor_copy(out=smod[:, cc : cc + 1], in_=pcol)
            nc.vector.tensor_mul(s2c, smod, smod)

            # wb[cin, cc, t, cout] = kernel * (s+1), cast to bf16
            wb = wpool.tile([128, 2, T, COUT], bf16)
            for cc in range(2):
                nc.vector.tensor_scalar_mul(wb[:, cc], kbase[:, cc], smod[:, cc : cc + 1])

            # sumsq[cout] = sum_cc s2c^T @ k2  -> [1, 256] in psum
            prow = psmall.tile([1, COUT], f32, tag="psmall")
            for cc in range(2):
                nc.tensor.matmul(
                    prow, s2c[:, cc : cc + 1], k2[:, cc], start=(cc == 0), stop=(cc == 1)
                )
            ssq_row = srow_pool.tile([1, COUT], f32)
            nc.vector.tensor_copy(out=ssq_row, in_=prow)

            sqc = dpool.tile([128, 2], f32)
            for oc in range(2):
                pcol2 = psmall.tile([128, 1], f32, tag="psmall")
                nc.tensor.matmul(
                    pcol2, ssq_row[:, oc * 128 : (oc + 1) * 128], ones1, start=True, stop=True
                )
                nc.scalar.activation(sqc[:, oc : oc + 1], pcol2, AF.Sqrt, bias=eps_sb)
            d_sb = dpool.tile([128, 2], f32)
            nc.vector.reciprocal(d_sb, sqc)

            # ---- x: load (cast bf16) + transpose to channel-major padded ----
            xpad = xpool.tile([128, 2, H + 2, PADW], bf16)
            for cc in range(2):
                nc.vector.memset(xpad[:, cc, 0, :], 0.0)
                nc.vector.memset(xpad[:, cc, H + 1, :], 0.0)
                nc.vector.memset(xpad[:, cc, :, 0], 0.0)
                nc.vector.memset(xpad[:, cc, :, PADW - 1], 0.0)

            for t8 in range(8):
                xtmp = xtpool.tile([128, 4, CIN], bf16)
                nc.gpsimd.dma_start(out=xtmp, in_=x_blk_ap(b, t8))
                for s in range(4):
                    blk = t8 * 4 + s  # 128-pixel block == image rows 2blk, 2blk+1
                    for cc in range(2):
                        pxt_t = pxt.tile([128, 2, 64], bf16)
                        nc.tensor.transpose(
                            pxt_t, xtmp[:, s, cc * 128 : (cc + 1) * 128], ident_b
                        )
                        nc.vector.tensor_copy(
                            out=xpad[:, cc, 2 * blk + 1 : 2 * blk + 3, 1 : W + 1],
                            in_=pxt_t,
                        )

            # ---- conv: 8 output row-tiles x 2 cout chunks x 18 matmuls ----
            for t8 in range(8):
                ob = obpool.tile([128, 4, COUT], f32)
                r0 = t8 * 8
                for oc in range(2):
                    ps = pconv.tile([128, 512], f32)
                    i = 0
                    for t in range(T):
                        dy, dx = t // 3 - 1, t % 3 - 1
                        for cc in range(2):
                            nc.tensor.matmul(
                                ps,
                                wb[:, cc, t, oc * 128 : (oc + 1) * 128],
                                xpad[:, cc, r0 + 1 + dy : r0 + 9 + dy, 1 + dx : W + 1 + dx],
                                start=(i == 0),
                                stop=(i == 17),
                            )
                            i += 1
                    o_sb = ospool.tile([128, 512], f32)
                    nc.scalar.activation(o_sb, ps, AF.Copy, scale=d_sb[:, oc : oc + 1])
                    for s in range(4):
                        pot_t = pot.tile([128, 128], f32)
                        nc.tensor.transpose(
                            pot_t, o_sb[:, s * 128 : (s + 1) * 128], ident_f
                        )
                        nc.vector.tensor_copy(
                            out=ob[:, s, oc * 128 : (oc + 1) * 128], in_=pot_t
                        )
                nc.sync.dma_start(out=y_blk_ap(b, t8), in_=ob)

    nc.compile()
    return nc


def _get_nc():
    if "nc" not in _CACHE:
        _CACHE["nc"] = _build_nc()
    return _CACHE["nc"]


def kernel(x, style, kernel, _trace=False):
    global LAST_EXEC_NS, LAST_MEAN_EXEC_NS
    from concourse.bass_utils import run_bass_kernel_spmd

    x = np.ascontiguousarray(x, dtype=np.float32)
    style = np.ascontiguousarray(style, dtype=np.float32)
    kern = np.ascontiguousarray(kernel, dtype=np.float32)

    nc = _get_nc()
    in_maps = [
        {
            "x": x[i * BPC : (i + 1) * BPC],
            "style": style[i * BPC : (i + 1) * BPC],
            "kernel": kern,
        }
        for i in range(NCORES)
    ]
    res = run_bass_kernel_spmd(nc, in_maps, core_ids=list(range(NCORES)), trace=_trace)
    LAST_EXEC_NS = res.exec_time_ns
    LAST_MEAN_EXEC_NS = res.mean_exec_time_ns
    return np.concatenate([res.results[i]["y"] for i in range(NCORES)], axis=0)


# revision 6
# speedup vs baseline: 1.0090x; 1.0090x over previous
"""Trainium2 Bass kernel for StyleGAN2-style modulated conv2d (ModConv2D).

Reference computation (per sample b):
    w      = kernel * (style[b] + 1)                 # modulate [3,3,Cin,Cout]
    w      = w / sqrt(sum(w^2, (kh,kw,Cin)) + 1e-8)  # demodulate per Cout
    y[b]   = conv2d_same(x[b], w)

Sharding: data-parallel over batch — 16 samples across 8 NeuronCores,
2 samples per core; the base kernel is replicated.

Device algorithm per core (2 samples):
  - conv as 9-tap accumulated matmuls: psum[cout,pix] += w[t,cin,cout]^T @
    xT[cin, pix+off], with x held zero-padded (66x66) channel-major in SBUF
    (bf16), weights modulated on-chip (bf16), accumulation in fp32.
  - demod factor d[cout] = rsqrt(sum_cin s^2 * K2 + 1e-8) in fp32 on device
    (K2 = sum_t kernel^2 precomputed once per core), applied as a
    per-partition scale when evicting psum.
  - x transposed to channel-major via PE transposes (4 per PSUM bank, one
    batched eviction copy); output transposed back to pixel-major the same
    way; I/O tensors keep the reference layouts in HBM.
"""

import numpy as np

B, H, W, CIN, COUT, KH, KW = 16, 64, 64, 256, 256, 3, 3
NCORES = 8
BPC = B // NCORES  # samples per core
T = KH * KW  # 9 taps
PADW = W + 2  # 66

_CACHE = {}
LAST_EXEC_NS = None
LAST_MEAN_EXEC_NS = None


def _build_nc():
    from contextlib import ExitStack

    import concourse.bacc as bacc
    import concourse.bass as bass
    import concourse.mybir as mybir
    import concourse.tile as tile
    from concourse.masks import make_identity

    f32 = mybir.dt.float32
    bf16 = mybir.dt.bfloat16
    AF = mybir.ActivationFunctionType

    nc = bacc.Bacc("TRN2", target_bir_lowering=False, debug=False)

    x_d = nc.dram_tensor("x", [BPC, H, W, CIN], f32, kind="ExternalInput")
    s_d = nc.dram_tensor("style", [BPC, CIN], f32, kind="ExternalInput")
    k_d = nc.dram_tensor("kernel", [KH, KW, CIN, COUT], f32, kind="ExternalInput")
    y_d = nc.dram_tensor("y", [BPC, H, W, COUT], f32, kind="ExternalOutput")

    XB = H * W * CIN  # x/y sample stride (elements)
    KKW = CIN * COUT  # kernel tap stride

    def x_blk_ap(b, t8):
        # [128 pix, 4 sblk, 256 cin] starting at pixel (t8*4)*128
        off = b * XB + t8 * 4 * 128 * CIN
        return bass.AP(x_d, off, [[CIN, 128], [128 * CIN, 4], [1, CIN]])

    def y_blk_ap(b, t8):
        off = b * XB + t8 * 4 * 128 * COUT
        return bass.AP(y_d, off, [[COUT, 128], [128 * COUT, 4], [1, COUT]])

    def k_cc_ap(cc):
        # [128 cin, 9 taps, 256 cout]
        return bass.AP(k_d, cc * 128 * COUT, [[COUT, 128], [KKW, T], [1, COUT]])

    with tile.TileContext(nc) as tc, ExitStack() as ctx:
        singles = ctx.enter_context(tc.tile_pool(name="singles", bufs=1))
        tmp_pool = ctx.enter_context(tc.tile_pool(name="tmp", bufs=1))
        wpool = ctx.enter_context(tc.tile_pool(name="wpool", bufs=2))
        dpool = ctx.enter_context(tc.tile_pool(name="dpool", bufs=2))
        srow_pool = ctx.enter_context(tc.tile_pool(name="srow", bufs=2))
        xpool = ctx.enter_context(tc.tile_pool(name="xpool", bufs=2))
        xtpool = ctx.enter_context(tc.tile_pool(name="xt", bufs=2 * 8))
        ospool = ctx.enter_context(tc.tile_pool(name="osb", bufs=3))
        obpool = ctx.enter_context(tc.tile_pool(name="ob", bufs=3))
        pconv = ctx.enter_context(tc.tile_pool(name="pconv", bufs=2, space="PSUM"))
        pxt = ctx.enter_context(tc.tile_pool(name="pxt", bufs=2, space="PSUM"))
        pot = ctx.enter_context(tc.tile_pool(name="pot", bufs=2, space="PSUM"))
        psmall = ctx.enter_context(tc.tile_pool(name="psmall", bufs=2, space="PSUM"))

        # style rows + base kernel first (conv weights are on the critical path)
        srows = []
        for b in range(BPC):
            srow = srow_pool.tile([1, CIN], f32, tag="srow")
            nc.sync.dma_start(out=srow, in_=s_d.ap()[b : b + 1, :])
            srows.append(srow)
        kbase = singles.tile([128, 2, T, COUT], f32)
        nc.sync.dma_start(out=kbase[:, 0], in_=k_cc_ap(0))
        nc.scalar.dma_start(out=kbase[:, 1], in_=k_cc_ap(1))

        # identities for PE transposes (gpsimd), before the x loads
        ident_b = singles.tile([128, 128], bf16)
        make_identity(nc, ident_b)
        ident_f = singles.tile([128, 128], f32)
        make_identity(nc, ident_f)

        # all x loads (cast fp32->bf16, SWDGE) issued upfront
        xts = []
        for b in range(BPC):
            row = []
            for t8 in range(8):
                xtmp = xtpool.tile([128, 4, CIN], bf16, tag="xtmp")
                nc.gpsimd.dma_start(out=xtmp, in_=x_blk_ap(b, t8))
                row.append(xtmp)
            xts.append(row)

        ones1 = singles.tile([1, 1], f32)
        nc.vector.memset(ones1, 1.0)
        eps_sb = singles.tile([128, 1], f32)
        nc.vector.memset(eps_sb, 1e-8)

        # K2[cin, cout] = sum_t kernel^2  (once per core)
        k2 = singles.tile([128, 2, COUT], f32)
        for cc in range(2):
            k2tmp = tmp_pool.tile([128, T, COUT], f32)
            nc.vector.tensor_mul(k2tmp, kbase[:, cc], kbase[:, cc])
            nc.vector.reduce_sum(
                out=k2[:, cc],
                in_=k2tmp.rearrange("p t c -> p c t"),
                axis=mybir.AxisListType.X,
            )

        for b in range(BPC):
            # ---- modulation factors ----
            srow1 = srow_pool.tile([1, CIN], f32, tag="srow1")
            nc.vector.tensor_scalar_add(srow1, srows[b], 1.0)

            smod = dpool.tile([128, 2], f32)  # (style+1) col-major per cc
            s2c = dpool.tile([128, 2], f32)
            for cc in range(2):
                pcol = psmall.tile([128, 1], f32, tag="psmall")
                nc.tensor.matmul(
                    pcol, srow1[:, cc * 128 : (cc + 1) * 128], ones1, start=True, stop=True
                )
                nc.vector.tensor_copy(out=smod[:, cc : cc + 1], in_=pcol)
            nc.vector.tensor_mul(s2c, smod, smod)

            # wb[cin, cc, t, cout] = kernel * (s+1), cast bf16 (split per tap
            # so the first conv matmuls unblock early)
            wb = wpool.tile([128, 2, T, COUT], bf16)
            for t in range(T):
                for cc in range(2):
                    nc.vector.tensor_scalar_mul(
                        wb[:, cc, t], kbase[:, cc, t], smod[:, cc : cc + 1]
                    )

            # sumsq[cout] = sum_cc s2c^T @ k2 -> [1, 256] -> demod d [128, 2]
            prow = psmall.tile([1, COUT], f32, tag="psmall")
            for cc in range(2):
                nc.tensor.matmul(
                    prow, s2c[:, cc : cc + 1], k2[:, cc], start=(cc == 0), stop=(cc == 1)
                )
            ssq_row = srow_pool.tile([1, COUT], f32, tag="ssq")
            nc.vector.tensor_copy(out=ssq_row, in_=prow)
            sqc = dpool.tile([128, 2], f32)
            for oc in range(2):
                pcol2 = psmall.tile([128, 1], f32, tag="psmall")
                nc.tensor.matmul(
                    pcol2, ssq_row[:, oc * 128 : (oc + 1) * 128], ones1, start=True, stop=True
                )
                nc.scalar.activation(sqc[:, oc : oc + 1], pcol2, AF.Sqrt, bias=eps_sb)
            d_sb = dpool.tile([128, 2], f32)
            nc.vector.reciprocal(d_sb, sqc)

            # ---- x transpose to channel-major padded + conv, interleaved ----
            xpad = xpool.tile([128, 2, H + 2, PADW], bf16)
            for cc in range(2):
                nc.vector.memset(xpad[:, cc, 0, :], 0.0)
                nc.vector.memset(xpad[:, cc, H + 1, :], 0.0)
                nc.vector.memset(xpad[:, cc, :, 0], 0.0)
                nc.vector.memset(xpad[:, cc, :, PADW - 1], 0.0)

            def transpose_block(t8):
                # xtmp [128 pix, 4, 256] -> xpad rows 8*t8 .. 8*t8+7
                xtmp = xts[b][t8]
                for cc in range(2):
                    pxt_t = pxt.tile([128, 8, 64], bf16, tag="pxt")
                    for s in range(4):
                        nc.tensor.transpose(
                            pxt_t[:, 2 * s : 2 * s + 2, :],
                            xtmp[:, s, cc * 128 : (cc + 1) * 128],
                            ident_b,
                        )
                    nc.vector.tensor_copy(
                        out=xpad[:, cc, 8 * t8 + 1 : 8 * t8 + 9, 1 : W + 1],
                        in_=pxt_t,
                    )

            def conv_tile(t8):
                # output rows 8*t8 .. 8*t8+7, both cout chunks
                ob = obpool.tile([128, 4, COUT], f32, tag="ob")
                r0 = t8 * 8
                for oc in range(2):
                    ps = pconv.tile([128, 512], f32, tag="pconv")
                    i = 0
                    for t in range(T):
                        dy, dx = t // 3 - 1, t % 3 - 1
                        for cc in range(2):
                            nc.tensor.matmul(
                                ps,
                                wb[:, cc, t, oc * 128 : (oc + 1) * 128],
                                xpad[:, cc, r0 + 1 + dy : r0 + 9 + dy, 1 + dx : W + 1 + dx],
                                start=(i == 0),
                                stop=(i == 17),
                            )
                            i += 1
                    o_sb = ospool.tile([128, 512], f32, tag="osb")
                    nc.scalar.activation(o_sb, ps, AF.Copy, scale=d_sb[:, oc : oc + 1])
                    pot_t = pot.tile([128, 4, 128], f32, tag="pot")
                    for s in range(4):
                        nc.tensor.transpose(
                            pot_t[:, s, :], o_sb[:, s * 128 : (s + 1) * 128], ident_f
                        )
                    nc.vector.tensor_copy(
                        out=ob[:, :, oc * 128 : (oc + 1) * 128], in_=pot_t
                    )
                nc.sync.dma_start(out=y_blk_ap(b, t8), in_=ob)

            transpose_block(0)
            for t8 in range(1, 8):
                transpose_block(t8)
                conv_tile(t8 - 1)
            conv_tile(7)

    nc.compile()
    return nc


def _get_nc():
    if "nc" not in _CACHE:
        _CACHE["nc"] = _build_nc()
    return _CACHE["nc"]


def kernel(x, style, kernel, _trace=False):
    global LAST_EXEC_NS, LAST_MEAN_EXEC_NS
    from concourse.bass_utils import run_bass_kernel_spmd

    x = np.ascontiguousarray(x, dtype=np.float32)
    style = np.ascontiguousarray(style, dtype=np.float32)
    kern = np.ascontiguousarray(kernel, dtype=np.float32)

    nc = _get_nc()
    in_maps = [
        {
            "x": x[i * BPC : (i + 1) * BPC],
            "style": style[i * BPC : (i + 1) * BPC],
            "kernel": kern,
        }
        for i in range(NCORES)
    ]
    res = run_bass_kernel_spmd(nc, in_maps, core_ids=list(range(NCORES)), trace=_trace)
    LAST_EXEC_NS = res.exec_time_ns
    LAST_MEAN_EXEC_NS = res.mean_exec_time_ns
    return np.concatenate([res.results[i]["y"] for i in range(NCORES)], axis=0)


# revision 8
# speedup vs baseline: 1.0845x; 1.0749x over previous
"""Trainium2 Bass kernel for StyleGAN2-style modulated conv2d (ModConv2D).

Reference computation (per sample b):
    w      = kernel * (style[b] + 1)                 # modulate [3,3,Cin,Cout]
    w      = w / sqrt(sum(w^2, (kh,kw,Cin)) + 1e-8)  # demodulate per Cout
    y[b]   = conv2d_same(x[b], w)

Sharding: data-parallel over batch — 16 samples across 8 NeuronCores,
2 samples per core; the base kernel is replicated.

Device algorithm per core (2 samples):
  - conv as 9-tap accumulated matmuls: psum[cout,pix] += w[t,cin,cout]^T @
    xT[cin, pix+off], with x held zero-padded (66x66) channel-major in SBUF
    (bf16), weights modulated on-chip (bf16), accumulation in fp32.
  - demod factor d[cout] = rsqrt(sum_cin s^2 * K2 + 1e-8) in fp32 on device
    (K2 = sum_t kernel^2 precomputed once per core), applied as a
    per-partition scale when evicting psum.
  - x transposed to channel-major via PE transposes (4 per PSUM bank, one
    batched eviction copy); output transposed back to pixel-major the same
    way; I/O tensors keep the reference layouts in HBM.
"""

import numpy as np

B, H, W, CIN, COUT, KH, KW = 16, 64, 64, 256, 256, 3, 3
NCORES = 8
BPC = B // NCORES  # samples per core
T = KH * KW  # 9 taps
PADW = W + 2  # 66

_CACHE = {}
LAST_EXEC_NS = None
LAST_MEAN_EXEC_NS = None


def _build_nc():
    from contextlib import ExitStack

    import concourse.bacc as bacc
    import concourse.bass as bass
    import concourse.mybir as mybir
    import concourse.tile as tile
    from concourse.masks import make_identity

    f32 = mybir.dt.float32
    bf16 = mybir.dt.bfloat16
    AF = mybir.ActivationFunctionType

    nc = bacc.Bacc("TRN2", target_bir_lowering=False, debug=False)

    x_d = nc.dram_tensor("x", [BPC, H, W, CIN], f32, kind="ExternalInput")
    s_d = nc.dram_tensor("style", [BPC, CIN], f32, kind="ExternalInput")
    k_d = nc.dram_tensor("kernel", [KH, KW, CIN, COUT], f32, kind="ExternalInput")
    y_d = nc.dram_tensor("y", [BPC, H, W, COUT], f32, kind="ExternalOutput")

    XB = H * W * CIN  # x/y sample stride (elements)
    KKW = CIN * COUT  # kernel tap stride

    def x_blk_ap(b, t8):
        # [128 pix, 4 sblk, 256 cin] starting at pixel (t8*4)*128
        off = b * XB + t8 * 4 * 128 * CIN
        return bass.AP(x_d, off, [[CIN, 128], [128 * CIN, 4], [1, CIN]])

    def y_blk_ap(b, t8):
        off = b * XB + t8 * 4 * 128 * COUT
        return bass.AP(y_d, off, [[COUT, 128], [128 * COUT, 4], [1, COUT]])

    def k_cc_ap(cc):
        # [128 cin, 9 taps, 256 cout]
        return bass.AP(k_d, cc * 128 * COUT, [[COUT, 128], [KKW, T], [1, COUT]])

    with tile.TileContext(nc) as tc, ExitStack() as ctx:
        singles = ctx.enter_context(tc.tile_pool(name="singles", bufs=1))
        tmp_pool = ctx.enter_context(tc.tile_pool(name="tmp", bufs=1))
        wpool = ctx.enter_context(tc.tile_pool(name="wpool", bufs=2))
        dpool = ctx.enter_context(tc.tile_pool(name="dpool", bufs=2))
        srow_pool = ctx.enter_context(tc.tile_pool(name="srow", bufs=2))
        xpool = ctx.enter_context(tc.tile_pool(name="xpool", bufs=2))
        xtpool = ctx.enter_context(tc.tile_pool(name="xt", bufs=2 * 8))
        ospool = ctx.enter_context(tc.tile_pool(name="osb", bufs=3))
        obpool = ctx.enter_context(tc.tile_pool(name="ob", bufs=3))
        pconv = ctx.enter_context(tc.tile_pool(name="pconv", bufs=2, space="PSUM"))
        pxt = ctx.enter_context(tc.tile_pool(name="pxt", bufs=2, space="PSUM"))
        pot = ctx.enter_context(tc.tile_pool(name="pot", bufs=2, space="PSUM"))
        psmall = ctx.enter_context(tc.tile_pool(name="psmall", bufs=2, space="PSUM"))

        # style rows + base kernel first (conv weights are on the critical path)
        srows = []
        for b in range(BPC):
            srow = srow_pool.tile([1, CIN], f32, tag="srow")
            nc.sync.dma_start(out=srow, in_=s_d.ap()[b : b + 1, :])
            srows.append(srow)
        kbase = singles.tile([128, 2, T, COUT], f32)
        nc.sync.dma_start(out=kbase[:, 0], in_=k_cc_ap(0))
        nc.scalar.dma_start(out=kbase[:, 1], in_=k_cc_ap(1))

        # identities for PE transposes (gpsimd), before the x loads
        ident_b = singles.tile([128, 128], bf16)
        make_identity(nc, ident_b)
        ident_f = singles.tile([128, 128], f32)
        make_identity(nc, ident_f)

        # all x loads (cast fp32->bf16, SWDGE) issued upfront
        xts = []
        for b in range(BPC):
            row = []
            for t8 in range(8):
                xtmp = xtpool.tile([128, 4, CIN], bf16, tag="xtmp")
                nc.gpsimd.dma_start(out=xtmp, in_=x_blk_ap(b, t8))
                row.append(xtmp)
            xts.append(row)

        ones1 = singles.tile([1, 1], f32)
        nc.vector.memset(ones1, 1.0)
        eps_sb = singles.tile([128, 1], f32)
        nc.vector.memset(eps_sb, 1e-8)

        # K2[cin, cout] = sum_t kernel^2  (once per core)
        k2 = singles.tile([128, 2, COUT], f32)
        for cc in range(2):
            k2tmp = tmp_pool.tile([128, T, COUT], f32)
            nc.vector.tensor_mul(k2tmp, kbase[:, cc], kbase[:, cc])
            nc.vector.reduce_sum(
                out=k2[:, cc],
                in_=k2tmp.rearrange("p t c -> p c t"),
                axis=mybir.AxisListType.X,
            )

        # ---- modulation + demod factors for BOTH samples, upfront ----
        wbs, dsbs = [], []
        for b in range(BPC):
            srow1 = srow_pool.tile([1, CIN], f32, tag="srow1")
            nc.vector.tensor_scalar_add(srow1, srows[b], 1.0)

            smod = dpool.tile([128, 2], f32)  # (style+1) col-major per cc
            s2c = dpool.tile([128, 2], f32)
            for cc in range(2):
                pcol = psmall.tile([128, 1], f32, tag="psmall")
                nc.tensor.matmul(
                    pcol, srow1[:, cc * 128 : (cc + 1) * 128], ones1, start=True, stop=True
                )
                nc.vector.tensor_copy(out=smod[:, cc : cc + 1], in_=pcol)
            nc.vector.tensor_mul(s2c, smod, smod)

            # wb[cin, cc, t, cout] = kernel * (s+1), cast bf16, on ACT (keeps
            # DVE free for the transpose-eviction copies)
            wb = wpool.tile([128, 2, T, COUT], bf16)
            for t in range(T):
                for cc in range(2):
                    nc.scalar.activation(
                        wb[:, cc, t], kbase[:, cc, t], AF.Copy,
                        scale=smod[:, cc : cc + 1],
                    )
            wbs.append(wb)

            # sumsq[cout] = sum_cc s2c^T @ k2 -> [1, 256] -> demod d [128, 2]
            prow = psmall.tile([1, COUT], f32, tag="psmall")
            for cc in range(2):
                nc.tensor.matmul(
                    prow, s2c[:, cc : cc + 1], k2[:, cc], start=(cc == 0), stop=(cc == 1)
                )
            ssq_row = srow_pool.tile([1, COUT], f32, tag="ssq")
            nc.vector.tensor_copy(out=ssq_row, in_=prow)
            sqc = dpool.tile([128, 2], f32)
            for oc in range(2):
                pcol2 = psmall.tile([128, 1], f32, tag="psmall")
                nc.tensor.matmul(
                    pcol2, ssq_row[:, oc * 128 : (oc + 1) * 128], ones1, start=True, stop=True
                )
                nc.scalar.activation(sqc[:, oc : oc + 1], pcol2, AF.Sqrt, bias=eps_sb)
            d_sb = dpool.tile([128, 2], f32)
            nc.vector.reciprocal(d_sb, sqc)
            dsbs.append(d_sb)

        for b in range(BPC):
            wb = wbs[b]
            d_sb = dsbs[b]
            # ---- x transpose to channel-major padded + conv, interleaved ----
            xpad = xpool.tile([128, 2, H + 2, PADW], bf16)
            for cc in range(2):
                nc.vector.memset(xpad[:, cc, 0, :], 0.0)
                nc.vector.memset(xpad[:, cc, H + 1, :], 0.0)
                nc.vector.memset(xpad[:, cc, :, 0], 0.0)
                nc.vector.memset(xpad[:, cc, :, PADW - 1], 0.0)

            def transpose_block(t8):
                # xtmp [128 pix, 4, 256] -> xpad rows 8*t8 .. 8*t8+7
                xtmp = xts[b][t8]
                for cc in range(2):
                    pxt_t = pxt.tile([128, 8, 64], bf16, tag="pxt")
                    for s in range(4):
                        nc.tensor.transpose(
                            pxt_t[:, 2 * s : 2 * s + 2, :],
                            xtmp[:, s, cc * 128 : (cc + 1) * 128],
                            ident_b,
                        )
                    nc.vector.tensor_copy(
                        out=xpad[:, cc, 8 * t8 + 1 : 8 * t8 + 9, 1 : W + 1],
                        in_=pxt_t,
                    )

            def conv_tile(t8):
                # output rows 8*t8 .. 8*t8+7, both cout chunks
                ob = obpool.tile([128, 4, COUT], bf16, tag="ob")
                r0 = t8 * 8
                for oc in range(2):
                    ps = pconv.tile([128, 512], f32, tag="pconv")
                    i = 0
                    for t in range(T):
                        dy, dx = t // 3 - 1, t % 3 - 1
                        for cc in range(2):
                            nc.tensor.matmul(
                                ps,
                                wb[:, cc, t, oc * 128 : (oc + 1) * 128],
                                xpad[:, cc, r0 + 1 + dy : r0 + 9 + dy, 1 + dx : W + 1 + dx],
                                start=(i == 0),
                                stop=(i == 17),
                            )
                            i += 1
                    o_sb = ospool.tile([128, 512], bf16, tag="osb")
                    nc.scalar.activation(o_sb, ps, AF.Copy, scale=d_sb[:, oc : oc + 1])
                    pot_t = pot.tile([128, 4, 128], bf16, tag="pot")
                    for s in range(4):
                        nc.tensor.transpose(
                            pot_t[:, s, :], o_sb[:, s * 128 : (s + 1) * 128], ident_b
                        )
                    nc.vector.tensor_copy(
                        out=ob[:, :, oc * 128 : (oc + 1) * 128], in_=pot_t
                    )
                nc.gpsimd.dma_start(out=y_blk_ap(b, t8), in_=ob)

            transpose_block(0)
            for t8 in range(1, 8):
                transpose_block(t8)
                conv_tile(t8 - 1)
            conv_tile(7)

    nc.compile()
    return nc


def _get_nc():
    if "nc" not in _CACHE:
        _CACHE["nc"] = _build_nc()
    return _CACHE["nc"]


def kernel(x, style, kernel, _trace=False):
    global LAST_EXEC_NS, LAST_MEAN_EXEC_NS
    from concourse.bass_utils import run_bass_kernel_spmd

    x = np.ascontiguousarray(x, dtype=np.float32)
    style = np.ascontiguousarray(style, dtype=np.float32)
    kern = np.ascontiguousarray(kernel, dtype=np.float32)

    nc = _get_nc()
    in_maps = [
        {
            "x": x[i * BPC : (i + 1) * BPC],
            "style": style[i * BPC : (i + 1) * BPC],
            "kernel": kern,
        }
        for i in range(NCORES)
    ]
    res = run_bass_kernel_spmd(nc, in_maps, core_ids=list(range(NCORES)), trace=_trace)
    LAST_EXEC_NS = res.exec_time_ns
    LAST_MEAN_EXEC_NS = res.mean_exec_time_ns
    return np.concatenate([res.results[i]["y"] for i in range(NCORES)], axis=0)
